# revision 19
# baseline (speedup 1.0000x reference)
"""Trainium2 Bass kernel for nn_GCNTime (GCN + per-t causal transformer over nodes).

Sharding: T=16 time steps across 8 cores (2 per core). The graph (dense
normalized adjacency) is replicated; every stage is independent across t,
so there are no collectives.

Per-core layout: activations are feature-major [h=128 partitions, token]
(token = t_local*2048 + node). All linear layers run with the weight as the
stationary matmul operand. GCN aggregation uses node-major x blocks as the
stationary operand against the transposed adjacency as the moving operand,
which yields feature-major output directly. Attention scores are computed
transposed (keys on partitions) so exp(scores) feeds the A@V matmul without
transposition; softmax denominators and layernorm statistics are
partition-axis reductions done with a ones-matrix matmul.

Engine balance: PSUM evacuations ride on ScalarE/VectorE; all SBUF->SBUF
elementwise work (LN affine tail, posenc adds) runs on GpSimd, which is
otherwise idle. conv_W/conv_b are pre-scaled by sqrt(H) on the host so the
positional-encoding add is a single tensor_scalar per destination.
"""

import math
from contextlib import ExitStack

import numpy as np
import ml_dtypes

import concourse.bacc as bacc
import concourse.tile as tile
from concourse import mybir
from concourse.bass_utils import run_bass_kernel_spmd

P = 128
N = 2048          # nodes
T = 16            # total time steps
TL = 2            # time steps per core
NB = N // P       # node blocks (16)
H = 128
DFF = 2048
NF = DFF // P     # ffn chunks (16)
L = 2
TOKS = TL * N     # tokens per core (4096)
NDC = N // 512    # 512-wide node chunks (4)
TC = TOKS // 512  # 512-wide token chunks (8)
EPS = 1e-5
SCALE = 1.0 / math.sqrt(H)
SQH = math.sqrt(H)

f32 = mybir.dt.float32
f32r = mybir.dt.float32r
bf16 = mybir.dt.bfloat16
bfnp = ml_dtypes.bfloat16

AF = mybir.ActivationFunctionType
ALU = mybir.AluOpType


def _emit(tc, io):
    nc = tc.nc
    with ExitStack() as ctx:
        consts = ctx.enter_context(tc.tile_pool(name="consts", bufs=1))
        spool = ctx.enter_context(tc.tile_pool(name="spool", bufs=5))
        xpool = ctx.enter_context(tc.tile_pool(name="xpool", bufs=1))
        resid = ctx.enter_context(tc.tile_pool(name="resid", bufs=1))
        actbf = ctx.enter_context(tc.tile_pool(name="actbf", bufs=2))
        qkvp = ctx.enter_context(tc.tile_pool(name="qkvp", bufs=2))
        apool = ctx.enter_context(tc.tile_pool(name="apool", bufs=2))
        ffp = ctx.enter_context(tc.tile_pool(name="ffp", bufs=4))
        small = ctx.enter_context(tc.tile_pool(name="small", bufs=4))
        hrp = ctx.enter_context(tc.tile_pool(name="hrp", bufs=2))
        ost = ctx.enter_context(tc.tile_pool(name="ost", bufs=2))
        psum = ctx.enter_context(tc.tile_pool(name="psum", bufs=8, space="PSUM"))

        dma = nc.sync.dma_start
        wdma = nc.gpsimd.dma_start
        adma = nc.scalar.dma_start

        # first adjacency tile on the sync queue before anything else: the
        # first aggregation matmuls depend on it
        s_tiles0 = []
        s0 = spool.tile([P, N], bf16, tag="s_tile", name="s0_0")
        dma(out=s0, in_=io["s_t"][0])
        s_tiles0.append(s0)

        # layer-0 node-major input: quarter DMAs on the scalar queue so the
        # first aggregation block lands quickly
        xnm = xpool.tile([P, TL * N], bf16, tag="xnm", name="xnm0_v2")
        xg = xnm.rearrange("p (t c f) -> p t c f", t=TL, c=NB)
        ig = io["x_nm"].rearrange("t (c p) f -> p t c f", p=P)
        for q in range(4):
            for t in range(TL):
                adma(out=xg[:, t, q * 4:(q + 1) * 4], in_=ig[:, t, q * 4:(q + 1) * 4])

        # ---- small constants on the gpsimd SWDGE queue ----
        ident_f = consts.tile([P, P], f32, tag="ident_f")
        wdma(out=ident_f, in_=io["ident_f"])
        ident_b = consts.tile([P, P], bf16, tag="ident_b")
        wdma(out=ident_b, in_=io["ident_b"])
        ones_b = consts.tile([P, P], bf16, tag="ones_b")
        wdma(out=ones_b, in_=io["ones_b"])
        tril_b = consts.tile([P, P], bf16, tag="tril_b")
        wdma(out=tril_b, in_=io["tril_b"])
        pe_t = consts.tile([P, TL], f32, tag="pe_t")
        wdma(out=pe_t, in_=io["pe_t"])
        eps_t = consts.tile([P, 1], f32, tag="eps_t")
        nc.vector.memset(eps_t, EPS)
        ones_f = consts.tile([P, P], f32r, tag="ones_f")
        wdma(out=ones_f, in_=io["ones_f"])

        # ---- weights on the gpsimd SWDGE queue (Pool idle early) ----
        wl = []
        for l in range(L):
            ldma = wdma
            d = {}
            d["convW"] = consts.tile([P, H], bf16, tag=f"convW{l}", name=f"convW{l}")
            ldma(out=d["convW"], in_=io["convW"][l])
            d["conv_b"] = consts.tile([P, 1], f32, tag=f"conv_b{l}", name=f"conv_b{l}")
            ldma(out=d["conv_b"], in_=io["conv_b"][l])
            d["wqkvT"] = consts.tile([P, 3 * H], bf16, tag=f"wqkvT{l}", name=f"wqkvT{l}")
            ldma(out=d["wqkvT"], in_=io["wqkvT"][l])
            d["bqkv"] = consts.tile([P, 3], f32, tag=f"bqkv{l}", name=f"bqkv{l}")
            ldma(out=d["bqkv"], in_=io["bqkv"][l])
            d["bqkv_vr"] = consts.tile([1, H], bf16, tag=f"bqkv_vr{l}", name=f"bqkv_vr{l}")
            ldma(out=d["bqkv_vr"], in_=io["bqkv_vr"][l])
            d["woT"] = consts.tile([P, H], bf16, tag=f"woT{l}", name=f"woT{l}")
            ldma(out=d["woT"], in_=io["woT"][l])
            d["bo"] = consts.tile([P, 1], f32, tag=f"bo{l}", name=f"bo{l}")
            ldma(out=d["bo"], in_=io["bo"][l])
            d["w1T"] = consts.tile([P, DFF], bf16, tag=f"w1T{l}", name=f"w1T{l}")
            ldma(out=d["w1T"], in_=io["w1T"][l])
            d["b1"] = consts.tile([P, NF], f32, tag=f"b1{l}", name=f"b1{l}")
            ldma(out=d["b1"], in_=io["b1"][l])
            d["w2T"] = consts.tile([P, NF, H], bf16, tag=f"w2T{l}", name=f"w2T{l}")
            ldma(out=d["w2T"], in_=io["w2T"][l].rearrange("c p h -> p c h"))
            d["b2"] = consts.tile([P, 1], f32, tag=f"b2{l}", name=f"b2{l}")
            ldma(out=d["b2"], in_=io["b2"][l])
            for nm in ("ln1g", "ln1b", "ln2g", "ln2b"):
                d[nm] = consts.tile([P, 1], f32, tag=f"{nm}{l}", name=f"{nm}{l}")
                ldma(out=d[nm], in_=io[nm][l])
            wl.append(d)

        for l in range(L):
            w = wl[l]

            # ================= GCN aggregation: agg[f, d] = sum_s x[s,f] * S[d,s]
            ps_agg = []
            for t in range(TL):
                for dc in range(NDC):
                    pa = psum.tile([P, 512], f32, tag="ps", name=f"agg{l}_{t}_{dc}")
                    ps_agg.append(pa)
            for c in range(NB):
                if l == 0 and c == 0:
                    s_tile = s_tiles0[0]
                else:
                    s_tile = spool.tile([P, N], bf16, tag="s_tile", name=f"s{l}_{c}")
                    dma(out=s_tile, in_=io["s_t"][c])
                for t in range(TL):
                    for dc in range(NDC):
                        nc.tensor.matmul(
                            ps_agg[t * NDC + dc],
                            xnm[:, (t * NB + c) * P:(t * NB + c + 1) * P],
                            s_tile[:, dc * 512:(dc + 1) * 512],
                            start=(c == 0), stop=(c == NB - 1),
                        )
            aggb = actbf.tile([P, TOKS], bf16, tag="aggb", bufs=1, name=f"aggb_v2_{l}")
            for t in range(TL):
                for dc in range(NDC):
                    osl = aggb[:, (t * NDC + dc) * 512:(t * NDC + dc + 1) * 512]
                    if dc % 2 == 0:
                        nc.scalar.copy(osl, ps_agg[t * NDC + dc])
                    else:
                        nc.vector.tensor_copy(osl, ps_agg[t * NDC + dc])

            # ================= GCN linear (W,b pre-scaled by sqrt(H)) + relu
            # -> hr, then +pe fanout: DVE writes f32 residual, Pool writes bf16
            h1 = resid.tile([P, TOKS], bf16, tag="h1", name=f"h1_{l}")
            for tch in range(TC):
                sl = slice(tch * 512, (tch + 1) * 512)
                t = tch // NDC
                pc = psum.tile([P, 512], f32, tag="ps", name=f"conv{l}_{tch}")
                nc.tensor.matmul(pc, w["convW"], aggb[:, sl], start=True, stop=True)
                hr = hrp.tile([P, 512], f32, tag="hr", name=f"hr{l}_{tch}")
                nc.scalar.activation(
                    out=hr, in_=pc, func=AF.Relu, bias=w["conv_b"])
                nc.vector.tensor_scalar(
                    out=h1[:, sl], in0=hr, scalar1=pe_t[:, t:t + 1], scalar2=None,
                    op0=ALU.add)

            # ================= attention: qkv for both t, then (ic, t)-interleaved
            z1 = resid.tile([P, TOKS], bf16, tag="z1", name=f"z1_{l}")
            qs, ks, vns, attns = [], [], [], []
            for t in range(TL):
                q_t = qkvp.tile([P, N], bf16, tag="q_t", name=f"q{l}_{t}")
                k_t = qkvp.tile([P, N], bf16, tag="k_t", name=f"k{l}_{t}")
                for part, dest in ((0, q_t), (1, k_t)):
                    for ncc in range(NDC):
                        sl = slice(ncc * 512, (ncc + 1) * 512)
                        pq = psum.tile([P, 512], f32, tag="ps", name=f"qkv{l}_{t}_{part}_{ncc}")
                        nc.tensor.matmul(
                            pq, w["wqkvT"][:, part * H:(part + 1) * H],
                            h1[:, t * N + ncc * 512: t * N + (ncc + 1) * 512],
                            start=True, stop=True)
                        nc.scalar.activation(
                            out=dest[:, sl], in_=pq, func=AF.Identity,
                            bias=w["bqkv"][:, part:part + 1])
                # v computed node-major directly: lhsT = h1b node-block
                # (stationary), rhs = Wv (moving); bias added as a rank-1
                # K=1 matmul accumulation. 4 node blocks share one psum bank,
                # evacuated by a single [P,512] copy.
                vn = qkvp.tile([P, NB, P], bf16, tag="vn", bufs=2, name=f"vn{l}_{t}")
                for jg in range(NB // 4):
                    pt = psum.tile([P, 512], f32, tag="ps", name=f"vtr{l}_{t}_{jg}")
                    for k4 in range(4):
                        j = jg * 4 + k4
                        hsl = h1[:, t * N + j * P: t * N + (j + 1) * P]
                        nc.tensor.matmul(pt[:, k4 * P:(k4 + 1) * P], hsl,
                                         w["wqkvT"][:, 2 * H:3 * H],
                                         start=(k4 == 0), stop=False)
                        nc.tensor.matmul(pt[:, k4 * P:(k4 + 1) * P], ones_b[0:1, :],
                                         w["bqkv_vr"], start=False, stop=(k4 == 3))
                    nc.vector.tensor_copy(
                        vn[:, jg * 4:(jg + 1) * 4, :],
                        pt.rearrange("p (c f) -> p c f", c=4))
                attnb = qkvp.tile([P, N], bf16, tag="attnb", bufs=2, name=f"attnb{l}_{t}")
                qs.append(q_t); ks.append(k_t); vns.append(vn); attns.append(attnb)

            # ============ fused chunk pipeline: attention -> LN1 -> FFN ->
            # LN2 -> transpose-out, skewed so PE (in-order) never waits on a
            # cross-engine chain it just started.
            yl1 = resid.tile([P, TOKS], bf16, tag="yl1", name=f"yl1_{l}")
            z2 = resid.tile([P, TOKS], bf16, tag="z2", name=f"z2_{l}")
            yo = resid.tile([P, TOKS], bf16, tag="yo", name=f"yo_{l}")
            if l < L - 1:
                xnm = xpool.tile([P, TL * N], bf16, tag="xnm", name=f"xnm{l + 1}")

            def emit_attn(ic, t):
                q_t, k_t, vn, attnb = qs[t], ks[t], vns[t], attns[t]
                jmax = 4 * ic + 4
                qsl = slice(ic * 512, (ic + 1) * 512)
                A = apool.tile([P, NB, 512], bf16, tag="A", name=f"A{l}_{t}_{ic}")
                # diagonal key-blocks (r>=0) only cover queries >= 128r in
                # this chunk: narrower matmuls/exp, and only the leading
                # 128x128 sub-block needs the per-element triangle mask
                def q0_of(j):
                    r = j - 4 * ic
                    return 128 * r if r > 0 else 0
                for j in range(jmax):
                    q0 = q0_of(j)
                    pa = psum.tile([P, 512], f32, tag="ps", name=f"sc{l}_{t}_{ic}_{j}")
                    nc.tensor.matmul(
                        pa[:, q0:], k_t[:, j * P:(j + 1) * P],
                        q_t[:, ic * 512 + q0:(ic + 1) * 512],
                        start=True, stop=True)
                    nc.scalar.activation(
                        out=A[:, j, q0:], in_=pa[:, q0:], func=AF.Exp, scale=SCALE)
                    if j - 4 * ic >= 0:
                        nc.vector.tensor_mul(A[:, j, q0:q0 + P],
                                             A[:, j, q0:q0 + P], tril_b)
                pd = psum.tile([P, 512], f32, tag="ps", name=f"dn{l}_{t}_{ic}")
                for j in range(jmax):
                    q0 = q0_of(j)
                    nc.tensor.matmul(pd[:, q0:], ones_b, A[:, j, q0:],
                                     start=(j == 0), stop=(j == jmax - 1))
                rec = small.tile([P, 512], f32, tag="rec", bufs=2, name=f"rec{l}_{t}_{ic}")
                nc.vector.reciprocal(rec, pd)
                pv = psum.tile([P, 512], f32, tag="ps", name=f"av{l}_{t}_{ic}")
                for j in range(jmax):
                    q0 = q0_of(j)
                    nc.tensor.matmul(pv[:, q0:], vn[:, j, :], A[:, j, q0:],
                                     start=(j == 0), stop=(j == jmax - 1))
                nc.vector.tensor_tensor(
                    out=attnb[:, qsl], in0=pv, in1=rec, op=ALU.mult)
                # Wo + bias + residual for this 512-token chunk
                po = psum.tile([P, 512], f32, tag="ps", name=f"wo{l}_{t}_{ic}")
                nc.tensor.matmul(po, w["woT"], attnb[:, qsl],
                                 start=True, stop=True)
                sl = slice(t * N + ic * 512, t * N + (ic + 1) * 512)
                nc.vector.scalar_tensor_tensor(
                    out=z1[:, sl], in0=po, scalar=w["bo"], in1=h1[:, sl],
                    op0=ALU.add, op1=ALU.add)

            def emit_ffn(tch):
                sl = slice(tch * 512, (tch + 1) * 512)
                p2 = psum.tile([P, 512], f32, tag="ps", name=f"ff2{l}_{tch}")
                for c in range(NF):
                    p1 = psum.tile([P, 512], f32, tag="ps", name=f"ff1{l}_{tch}_{c}")
                    nc.tensor.matmul(p1, w["w1T"][:, c * P:(c + 1) * P],
                                     yl1[:, sl], start=True, stop=True)
                    f1 = ffp.tile([P, 512], bf16, tag="f1", name=f"f1_{l}_{tch}_{c}")
                    if c % 2 == 0:
                        nc.scalar.activation(
                            out=f1, in_=p1, func=AF.Relu, bias=w["b1"][:, c:c + 1])
                    else:
                        nc.vector.tensor_scalar(
                            out=f1, in0=p1, scalar1=w["b1"][:, c:c + 1], scalar2=0.0,
                            op0=ALU.add, op1=ALU.max)
                    nc.tensor.matmul(p2, w["w2T"][:, c, :], f1,
                                     start=(c == 0), stop=(c == NF - 1))
                nc.vector.scalar_tensor_tensor(
                    out=z2[:, sl], in0=p2, scalar=w["b2"], in1=yl1[:, sl],
                    op0=ALU.add, op1=ALU.add)

            def emit_tr(tch):
                # transpose one LN2 chunk (= one 4-block group) out of yo
                t, g = tch // NDC, tch % NDC
                pt = psum.tile([P, 512], bf16, tag="ps", name=f"tr{l}_{t}_{g}")
                for k in range(4):
                    c = g * 4 + k
                    nc.tensor.transpose(
                        pt[:, k * P:(k + 1) * P],
                        yo[:, (t * NB + c) * P:(t * NB + c + 1) * P],
                        ident_b)
                if l < L - 1:
                    nc.vector.tensor_copy(
                        xnm[:, (t * NB + g * 4) * P:(t * NB + g * 4 + 4) * P], pt)
                else:
                    stg = ost.tile([P, 4, P], f32, tag="stg", name=f"stg{t}_{g}")
                    nc.vector.tensor_copy(stg, pt.rearrange("p (c f) -> p c f", c=4))
                    dma(out=io["y"][t].rearrange("(g c p) f -> g p c f", g=4, c=4)[g],
                        in_=stg)

            # pipeline stages, skewed: chunk k's FFN runs while chunk k+1's
            # attention occupies PE; LN2 one further behind; transposes last.
            done1, done2 = [], []
            for ic in range(NDC):
                for t in range(TL):
                    emit_attn(ic, t)
                    tch = t * NDC + ic
                    ln_chunk(tc, psum, small, ones_b, ones_f, eps_t, z1, tch, yl1, None,
                             w["ln1g"], w["ln1b"], f"ln1_{l}")
                    done1.append(tch)
                    if len(done1) >= 2:
                        c2 = done1[-2]
                        emit_ffn(c2)
                        ln_chunk(tc, psum, small, ones_b, ones_f, eps_t, z2, c2, yo, None,
                                 w["ln2g"], w["ln2b"], f"ln2_{l}")
                        done2.append(c2)
                    if len(done2) >= 2:
                        emit_tr(done2[-2])
            for c2 in (done1[-1],):
                emit_ffn(c2)
                ln_chunk(tc, psum, small, ones_b, ones_f, eps_t, z2, c2, yo, None,
                         w["ln2g"], w["ln2b"], f"ln2_{l}", tail_on_dve=True)
                done2.append(c2)
            emit_tr(done2[-2])
            emit_tr(done2[-1])


def ln_chunk(tc, psum, small, ones_b, ones_f, eps_t, z, tch, yout, youtb, g_ap,
             b_ap, nm, tail_on_dve=False):
    """LayerNorm over the partition (feature) axis of one 512-token chunk.

    Stats via fp32r ones-matmul partition reduction (full-rate at N=512).
    rstd comes from exp(-0.5*ln(P^2*var + P^2*eps)) so every ACT function
    used in the kernel (Square/Ln/Exp/Relu/Identity/Copy) lives in ONE
    activation table ('natural_log_exp_and_others') -- no table reloads.
    The leftover 1/P factor is folded into gamma on the host (g' = g*P).
    The affine tail runs on GpSimd so the PSUM-evacuating engines stay free.
    """
    nc = tc.nc
    sl = slice(tch * 512, (tch + 1) * 512)
    sq = small.tile([P, 512], f32r, tag="lnbf", bufs=3, name=f"sq_{nm}_{tch}")
    nc.scalar.activation(out=sq, in_=z[:, sl], func=AF.Square)
    p1 = psum.tile([P, 512], f32, tag="ps", name=f"lns_{nm}_{tch}")
    nc.tensor.matmul(p1, ones_b, z[:, sl], start=True, stop=True)
    p2 = psum.tile([P, 512], f32, tag="ps", name=f"lnq_{nm}_{tch}")
    nc.tensor.matmul(p2, ones_f, sq, start=True, stop=True)
    # zc = z - sum(z)/P   (reads the sum psum directly)
    zc = small.tile([P, 512], f32, tag="lntmp", bufs=6, name=f"zc_{nm}_{tch}")
    nc.vector.scalar_tensor_tensor(
        out=zc, in0=p1, scalar=-1.0 / P, in1=z[:, sl],
        op0=ALU.mult, op1=ALU.add)
    # varq: mean^2 -> var; rstd = exp(-0.5 * ln(var + eps)). Square/Ln/Exp
    # all live in the pinned activation table, so no table reloads.
    varq = small.tile([P, 512], f32, tag="lntmp", bufs=6, name=f"varq_{nm}_{tch}")
    nc.scalar.activation(out=varq, in_=p1, func=AF.Square, scale=1.0 / P)
    nc.vector.scalar_tensor_tensor(
        out=varq, in0=p2, scalar=1.0 / P, in1=varq,
        op0=ALU.mult, op1=ALU.subtract)
    nc.scalar.activation(out=varq, in_=varq, func=AF.Ln, bias=eps_t)
    nc.scalar.activation(out=varq, in_=varq, func=AF.Exp, scale=-0.5)
    zcv = small.tile([P, 512], f32, tag="lntmp", bufs=6, name=f"zcv_{nm}_{tch}")
    nc.vector.tensor_tensor(out=zcv, in0=zc, in1=varq, op=ALU.mult)
    eng = nc.vector if tail_on_dve else nc.gpsimd
    eng.tensor_scalar(
        out=yout[:, sl], in0=zcv, scalar1=g_ap, scalar2=b_ap,
        op0=ALU.mult, op1=ALU.add)
    if youtb is not None:
        nc.gpsimd.tensor_scalar(
            out=youtb[:, sl], in0=zcv, scalar1=g_ap, scalar2=b_ap,
            op0=ALU.mult, op1=ALU.add)


_CACHE = {}


def _pin_act_table(arch):
    """Constrain the activation-table chooser to 'natural_log_exp_and_others'
    for Exp/Ln so the whole kernel (Exp/Ln/Relu/Square/Identity/Copy) runs
    from one table with no mid-kernel reloads. Only compile-time selection
    changes; the table ids and runtime contents are untouched."""
    from concourse.hw_specs import get_activation_tables
    tabs = get_activation_tables(arch)
    for name, s in tabs.items():
        if name != "natural_log_exp_and_others":
            s.discard(AF.Exp)
            s.discard(AF.Ln)


def _build():
    if "nc" in _CACHE:
        return _CACHE["nc"], _CACHE["io_names"]
    nc = bacc.Bacc("TRN2", target_bir_lowering=False, debug=False, num_devices=8)
    _pin_act_table(nc.m.arch)
    io = {}

    def inp(name, shape, dt):
        io[name] = nc.dram_tensor(name, shape, dt, kind="ExternalInput").ap()

    inp("x_nm", [TL, N, H], bf16)
    inp("s_t", [NB, P, N], bf16)
    inp("pe_t", [P, TL], f32)
    inp("convW", [L, P, H], bf16)
    inp("conv_b", [L, P, 1], f32)
    inp("wqkvT", [L, P, 3 * H], bf16)
    inp("bqkv", [L, P, 3], f32)
    inp("bqkv_vr", [L, 1, H], bf16)
    inp("woT", [L, P, H], bf16)
    inp("bo", [L, P, 1], f32)
    inp("w1T", [L, P, DFF], bf16)
    inp("b1", [L, P, NF], f32)
    inp("w2T", [L, NF, P, H], bf16)
    inp("b2", [L, P, 1], f32)
    inp("ln1g", [L, P, 1], f32)
    inp("ln1b", [L, P, 1], f32)
    inp("ln2g", [L, P, 1], f32)
    inp("ln2b", [L, P, 1], f32)
    inp("tril_b", [P, P], bf16)
    inp("ident_f", [P, P], f32)
    inp("ident_b", [P, P], bf16)
    inp("ones_b", [P, P], bf16)
    inp("ones_f", [P, P], f32r)
    inp("vtag", [1, 27], f32)
    io["y"] = nc.dram_tensor("y", [TL, N, H], f32, kind="ExternalOutput").ap()

    with tile.TileContext(nc) as t:
        _emit(t, io)
    nc.compile()
    _CACHE["nc"] = nc
    _CACHE["io_names"] = list(io)
    return nc, list(io)


def _host_prep(inputs):
    """Build the shared (replicated) device arrays from the full inputs."""
    x = np.asarray(inputs["x"], np.float32)
    edge = np.asarray(inputs["edge_index"])

    src = np.concatenate([edge[0], np.arange(N, dtype=edge.dtype)])
    dst = np.concatenate([edge[1], np.arange(N, dtype=edge.dtype)])
    deg = np.zeros(N, np.float32)
    np.add.at(deg, dst, 1.0)
    dinv = 1.0 / np.sqrt(deg)
    normv = (dinv[src] * dinv[dst]).astype(np.float32)
    S = np.zeros((N, N), np.float32)
    np.add.at(S, (dst, src), normv)
    s_t = np.ascontiguousarray(S.T.reshape(NB, P, N)).astype(bfnp)

    pos = np.arange(T, dtype=np.float32)[:, None]
    ii = np.arange(0, H, 2, dtype=np.float32)
    pes = np.sin(pos / (10000.0 ** (2.0 * ii / H))).astype(np.float32)
    pec = np.cos(pos / (10000.0 ** (2.0 * (ii + 1.0) / H))).astype(np.float32)
    pe = np.stack([pes, pec], axis=-1).reshape(T, H).astype(np.float32)

    conv_W = np.asarray(inputs["conv_W"], np.float32)
    Wqkv = np.asarray(inputs["Wqkv"], np.float32)
    Wo = np.asarray(inputs["Wo"], np.float32)
    W1 = np.asarray(inputs["W1"], np.float32)
    W2 = np.asarray(inputs["W2"], np.float32)

    shared = {
        "s_t": s_t,
        # conv weight/bias pre-scaled by sqrt(H): relu(x@W + b)*sqrt(H)
        # == relu(x@(W*sqrt(H)) + b*sqrt(H))
        "convW": (conv_W * SQH).astype(bfnp),
        "conv_b": (np.asarray(inputs["conv_b"], np.float32) * SQH).reshape(L, P, 1),
        "wqkvT": np.ascontiguousarray(Wqkv.transpose(0, 2, 1)).astype(bfnp),
        "bqkv": np.ascontiguousarray(
            np.asarray(inputs["bqkv"], np.float32).reshape(L, 3, P).transpose(0, 2, 1)),
        "bqkv_vr": np.asarray(inputs["bqkv"], np.float32).reshape(
            L, 3, P)[:, 2:3, :].astype(bfnp),
        "woT": np.ascontiguousarray(Wo.transpose(0, 2, 1)).astype(bfnp),
        "bo": np.asarray(inputs["bo"], np.float32).reshape(L, P, 1),
        "w1T": np.ascontiguousarray(W1.transpose(0, 2, 1)).astype(bfnp),
        "b1": np.ascontiguousarray(
            np.asarray(inputs["b1"], np.float32).reshape(L, NF, P).transpose(0, 2, 1)),
        "w2T": np.ascontiguousarray(
            W2.transpose(0, 2, 1).reshape(L, NF, P, H)).astype(bfnp),
        "b2": np.asarray(inputs["b2"], np.float32).reshape(L, P, 1),
        "ln1g": np.asarray(inputs["ln1_g"], np.float32).reshape(L, P, 1),
        "ln1b": np.asarray(inputs["ln1_b"], np.float32).reshape(L, P, 1),
        "ln2g": np.asarray(inputs["ln2_g"], np.float32).reshape(L, P, 1),
        "ln2b": np.asarray(inputs["ln2_b"], np.float32).reshape(L, P, 1),
        "tril_b": (np.arange(P)[None, :] >= np.arange(P)[:, None]).astype(bfnp),
        "ident_f": np.eye(P, dtype=np.float32),
        "ident_b": np.eye(P, dtype=np.float32).astype(bfnp),
        "ones_b": np.ones((P, P), np.float32).astype(bfnp),
        "ones_f": np.ones((P, P), np.float32),
        "vtag": np.zeros((1, 27), np.float32),
    }
    return shared, x, pe


def kernel(**inputs):
    nc, _ = _build()
    shared, x, pe = _host_prep(inputs)

    in_maps = []
    for core in range(8):
        t0 = core * TL
        m = dict(shared)
        m["x_nm"] = np.ascontiguousarray(
            x[:, t0:t0 + TL, :].transpose(1, 0, 2)).astype(bfnp)
        m["pe_t"] = np.ascontiguousarray(pe[t0:t0 + TL].T)
        in_maps.append(m)

    res = run_bass_kernel_spmd(nc, in_maps, list(range(8)))

    out = np.zeros((N, T, H), np.float32)
    for core in range(8):
        t0 = core * TL
        out[:, t0:t0 + TL, :] = res.results[core]["y"].transpose(1, 0, 2)
    return out
